# revision 8
# baseline (speedup 1.0000x reference)
"""CodeSage attention (B=2, S=2048, H=1024, 16 heads x 64) on 8 Trainium2 cores.

Sharding: tensor-parallel over heads — 2 heads per core. Each core computes
its head-group's QKV projection, attention, and the c_proj partial product;
the host sums the 8 partials and adds c_proj_b + bv @ c_proj_w (the V-bias
contribution reduces to a constant output row, applied host-side exactly).

Device-side design (bf16 matmuls, fp32 accumulation):

phase 1 (k-outer for stationary reuse, two 4-block passes so the projection
accumulators only occupy 4 PSUM banks and the score pool can coexist):
    qT,kT [128=2*64, T] = Wslice^T @ hsT   (1/sqrt(hd) folded into wq host-side)
    V natural per key tile; the PSUM->aug copy fuses the exp(mask) row scale:
    aug[g] = [ v'_h0(64) | em64(64) | v'_h1(64) ]  where em = exp(mask),
    v' = v*em — this folds the additive mask into V/ones stationaries, making
    the exp bias-free and the sumexp mask-aware.

phase 2, per (batch, 512-query-block) i — software-pipelined and interleaved
so ScalarE (the exp pacer) never starves:
    A: scoresT[sk,sq] = kT-slices^T @ qT  (2-head row-tiled pairs, K=64)
       probs = exp(scoresT)               (ScalarE)
    B: aug matmuls (K=128): psA = [ctx_h0 | se_h0*64], psB = [se_h1*64 | ctx_h1]
    N: DVE copy + DMA realign of sumexp rows, approx-reciprocal, ctxn = ctx*rec
    C: c_proj partial: out_tile = ctxn^T @ wp -> DVE copy -> DMA out
    Emission: Q, K, A(0), V; then for i: ILV[B(i) ~ A(i+1)], N(i), C(i-1).
"""

import numpy as np
import ml_dtypes

B, S, H = 2, 2048, 1024
NH, HD = 16, 64
NCORES = 8
HPC = NH // NCORES          # heads per core = 2
DC = HPC * HD               # per-core head dims = 128
T = B * S                   # 4096 tokens
KC = H // 128               # 8 contraction chunks
NBLK = T // 512             # 8 column blocks of 512 tokens
SQB = S // 512              # 4 query blocks per batch
SKT = S // 128              # 16 key tiles per batch
NIT = B * SQB               # 8 pipelined phase-2 iterations

_CACHE = {}


def _build_nc():
    import concourse.mybir as mybir
    import concourse.tile as tile
    from concourse import bacc

    f32 = mybir.dt.float32
    bf16 = mybir.dt.bfloat16

    nc = bacc.Bacc("TRN2", target_bir_lowering=False, debug=False,
                   num_devices=NCORES)

    hsT_d = nc.dram_tensor("hsT", [H, T], bf16, kind="ExternalInput")
    wq_d = nc.dram_tensor("wq", [128, KC, DC], bf16, kind="ExternalInput")
    wk_d = nc.dram_tensor("wk", [128, KC, DC], bf16, kind="ExternalInput")
    wv_d = nc.dram_tensor("wv", [128, KC, DC], bf16, kind="ExternalInput")
    wp_d = nc.dram_tensor("wp", [DC, H], bf16, kind="ExternalInput")
    bq_d = nc.dram_tensor("bq", [DC, 1], f32, kind="ExternalInput")
    bk_d = nc.dram_tensor("bk", [DC, 1], f32, kind="ExternalInput")
    mask_d = nc.dram_tensor("mask", [B, S], f32, kind="ExternalInput")
    out_d = nc.dram_tensor("out", [T, H], bf16, kind="ExternalOutput")

    i32 = mybir.dt.int32
    EXP = mybir.ActivationFunctionType.Exp
    MULT = mybir.AluOpType.mult
    ADDOP = mybir.AluOpType.add
    # 2-term phase-averaged bitcast exp: p = F(round(s*A+B)) + F(round(s*A+B)
    # - 2^22) with F = int32 bitcast to f32. Sawtooth rel err within +-1.05%
    # (rms 0.7%); applied to FAST_MOD'th score tiles only, so the overall
    # output perturbation stays ~0.3%. Offloads exp columns from ScalarE
    # (the phase-2 pacer) onto DVE (1 op) + GpSimd (2 ops).
    SCH_A = 12102203.161561485          # 2^23 * log2(e)
    SCH_B = 1058453365.9787611          # 2^23 * (127 - log2(1.76850))
    FAST_SET = (2, 7, 12)               # ILV tile indices on the fast path

    with tile.TileContext(nc) as tc:
        with (
            tc.tile_pool(name="const", bufs=1) as cpool,
            tc.tile_pool(name="qkv", bufs=1) as qpool,
            tc.tile_pool(name="probs", bufs=27) as ppool,
            tc.tile_pool(name="ctxn", bufs=2) as npool,
            tc.tile_pool(name="rcin", bufs=2) as ripool,
            tc.tile_pool(name="rec", bufs=2) as rpool,
            tc.tile_pool(name="ob", bufs=3) as opool,
            tc.tile_pool(name="schint", bufs=2) as ipool,
            tc.tile_pool(name="ps_sc", bufs=2, space="PSUM") as pssc,
        ):
            wq_sb = cpool.tile([128, KC, DC], bf16)
            wk_sb = cpool.tile([128, KC, DC], bf16)
            wv_sb = cpool.tile([128, KC, DC], bf16)
            wp_sb = cpool.tile([DC, H], bf16)
            bq_sb = cpool.tile([DC, 1], f32)
            bk_sb = cpool.tile([DC, 1], f32)
            ones64 = cpool.tile([128, 64], bf16)
            mask_sb = cpool.tile([128, B, SKT], f32)
            em_sb = cpool.tile([128, B, SKT], f32)

            # ramp-critical DMAs only: wk+wq gate the first proj wave;
            # everything else is issued after the hs quarter-0 transfers.
            nc.sync.dma_start(wk_sb[:], wk_d.ap())
            nc.sync.dma_start(wq_sb[:], wq_d.ap())
            nc.vector.memset(ones64[:], 1.0)

            qT_sb = qpool.tile([128, T], bf16)   # rows 0:64 head0, 64:128 head1
            kT_sb = qpool.tile([128, T], bf16)
            # aug stationaries: per key tile g, [v'_h0 | em64 | v'_h1]
            aug_sb = qpool.tile([128, B * SKT, 192], bf16)

            # ---- phase-2 emission helpers ---------------------------------
            def emit_sc(i, skt, fast=False):
                b = i // SQB
                sq0 = b * S + (i % SQB) * 512
                sk = slice(b * S + skt * 128, b * S + skt * 128 + 128)
                sq = slice(sq0, sq0 + 512)
                sc_ps = pssc.tile([128, 1024], f32, tag="sc", name="sc_ps")
                nc.tensor.matmul(sc_ps[:, 0:512], lhsT=kT_sb[0:64, sk],
                                 rhs=qT_sb[0:64, sq], start=True, stop=True,
                                 skip_group_check=True)
                nc.tensor.matmul(sc_ps[:, 512:1024], lhsT=kT_sb[64:128, sk],
                                 rhs=qT_sb[64:128, sq], start=True, stop=True,
                                 skip_group_check=True)
                pr = ppool.tile([128, 1024], bf16, tag="pr", name="pr")
                if fast:
                    ia = ipool.tile([128, 1024], i32, tag="ia", name="ia")
                    ib = ipool.tile([128, 1024], i32, tag="ib", name="ib")
                    nc.vector.tensor_scalar(ia[:], sc_ps[:], SCH_A, SCH_B,
                                            op0=MULT, op1=ADDOP)
                    nc.gpsimd.tensor_scalar_add(ib[:], ia[:], -4194304)
                    nc.gpsimd.tensor_tensor(pr[:], ia[:].bitcast(f32),
                                            ib[:].bitcast(f32), op=ADDOP)
                else:
                    nc.scalar.activation(pr[:], sc_ps[:], EXP)
                return pr

            # ---- phase 1: QKV projection ----------------------------------
            with (
                tc.tile_pool(name="hs", bufs=1) as hpool,
                tc.tile_pool(name="ps1", bufs=1, space="PSUM") as ps1,
            ):
                hs_all = hpool.tile([128, KC, T], bf16)
                # quarter-chunk transfers, quarter-major: the first proj pass
                # (kT/qT blocks 0-1) only reads column-quarter 0 of every
                # chunk, so emitting all chunks' quarter-0 first lets scores
                # start after ~2 MB of input instead of all 8 MB.
                for q4 in range(4):
                    cs = slice(q4 * 1024, (q4 + 1) * 1024)
                    for k in range(KC):
                        # alternate issue queues: halves the serial DMA-issue
                        # ramp (sync also carries wk/wq ahead of chunk k=0)
                        eng = nc.sync if k % 2 == 0 else nc.gpsimd
                        eng.dma_start(hs_all[:, k, cs],
                                      hsT_d.ap()[k * 128:(k + 1) * 128, cs])
                    if q4 == 0:
                        # deferred non-ramp-critical inputs, by first use:
                        # biases at the first proj writes, mask/em at the V
                        # phase, wp only at cproj
                        nc.sync.dma_start(bq_sb[:], bq_d.ap())
                        nc.sync.dma_start(bk_sb[:], bk_d.ap())
                        nc.sync.dma_start(
                            mask_sb[:],
                            mask_d.ap().rearrange("b (t p) -> p b t", p=128))
                        nc.scalar.activation(em_sb[:], mask_sb[:], EXP)
                    elif q4 == 1:
                        nc.gpsimd.dma_start(wv_sb[:], wv_d.ap())
                    elif q4 == 2:
                        nc.gpsimd.dma_start(wp_sb[:], wp_d.ap())

                def proj_joint(lanes):
                    """k-major accumulation over several (w, dst, bias, blk)
                    lanes at once — paces with arriving hs chunks."""
                    ps = [ps1.tile([128, 512], f32, tag=f"b{j}", name=f"ps_j{j}")
                          for j in range(len(lanes))]
                    for k in range(KC):
                        for j, (w_sb, _, _, blk) in enumerate(lanes):
                            cols = slice(blk * 512, (blk + 1) * 512)
                            nc.tensor.matmul(ps[j][:], lhsT=w_sb[:, k, :],
                                             rhs=hs_all[:, k, cols],
                                             start=(k == 0), stop=(k == KC - 1),
                                             skip_group_check=True)
                    for j, (_, dst_sb, bias_sb, blk) in enumerate(lanes):
                        cols = slice(blk * 512, (blk + 1) * 512)
                        nc.vector.tensor_scalar_add(dst_sb[:, cols], ps[j][:],
                                                    bias_sb[:, 0:1])

                KL = (wk_sb, kT_sb, bk_sb)
                QL = (wq_sb, qT_sb, bq_sb)
                # batch-0 K and Q first; exp for iteration 0 starts as soon as
                # the needed kT/qT halves exist, while PE continues projecting.
                proj_joint([KL + (0,), KL + (1,), QL + (0,)])
                probs0 = [emit_sc(0, skt) for skt in range(SKT // 2)]
                proj_joint([KL + (2,), KL + (3,), QL + (1,)])
                probs0 += [emit_sc(0, skt) for skt in range(SKT // 2, SKT)]
                proj_joint([QL + (2,), QL + (3,)])
                proj_joint([KL + (4,), KL + (5,), QL + (4,), QL + (5,)])
                proj_joint([KL + (6,), KL + (7,), QL + (6,), QL + (7,)])

                # V natural per key tile g; PSUM->aug copy fuses em scaling.
                # Bridge pairs of iteration-1 scores/exp into the V loop so
                # ScalarE doesn't starve between A(0) and the first ILV.
                # em columns for all 32 tiles in one broadcast op (em is
                # per-(partition, tile); replicate across the 64 cols)
                em_bc = em_sb[:].rearrange("p b t -> p (b t)").unsqueeze(2)
                nc.vector.tensor_scalar_mul(
                    aug_sb[:, :, 64:128], em_bc.broadcast_to([128, B * SKT, 64]),
                    1.0)
                bridge = []
                for g in range(B * SKT):
                    b, skt = g // SKT, g % SKT
                    em = em_sb[:, b, skt:skt + 1]
                    gc = slice(g * 128, (g + 1) * 128)
                    v_ps = ps1.tile([128, DC], f32, tag=f"b{g % 4}", name="v_ps")
                    for k in range(KC):
                        nc.tensor.matmul(v_ps[:], lhsT=hs_all[:, k, gc],
                                         rhs=wv_sb[:, k, :],
                                         start=(k == 0), stop=(k == KC - 1),
                                         skip_group_check=True)
                    # both 64-wide v halves in one op: out cols {0:64,128:192}
                    aug_v = aug_sb[:, g, :].rearrange("p (a b) -> p a b", a=3)[:, 0:3:2, :]
                    src_v = v_ps[:].rearrange("p (two c) -> p two c", two=2)
                    nc.vector.tensor_scalar_mul(aug_v, src_v, em)
                    if NIT > 1 and g % 3 == 2 and len(bridge) < 10:
                        bridge.append(emit_sc(1, len(bridge)))
                        bridge.append(emit_sc(1, len(bridge)))

            # ---- phase 2: attention + c_proj ------------------------------
            with tc.tile_pool(name="ps_ab", bufs=2, space="PSUM") as psab:
                prev = None  # (ctxn tile, sq0) pending c_proj

                def emit_cproj(ctxn, sq0, final=False):
                    # output DMA rides the idle GpSimd queue so it cannot
                    # delay the sync-queue rec_in realign DMAs (normalize
                    # critical path)
                    for t4 in range(4):
                        tok = slice(t4 * 128, (t4 + 1) * 128)
                        rows = slice(sq0 + t4 * 128, sq0 + (t4 + 1) * 128)
                        op_a = psab.tile([128, 512], f32, tag="pa", name="op_a")
                        op_b = psab.tile([128, 512], f32, tag="pb", name="op_b")
                        nc.tensor.matmul(op_a[:], lhsT=ctxn[:, tok],
                                         rhs=wp_sb[:, 0:512], start=True, stop=True,
                                         skip_group_check=True)
                        nc.tensor.matmul(op_b[:], lhsT=ctxn[:, tok],
                                         rhs=wp_sb[:, 512:1024], start=True, stop=True,
                                         skip_group_check=True)
                        ob = opool.tile([128, 1024], bf16, tag="ob", name="ob")
                        nc.vector.tensor_copy(ob[:, 0:512], op_a[:])
                        if final:
                            # exp stream is over: ScalarE is idle, split the
                            # PSUM evacuation so the tail isn't DVE-serial
                            nc.scalar.copy(ob[:, 512:1024], op_b[:])
                        else:
                            nc.vector.tensor_copy(ob[:, 512:1024], op_b[:])
                        nc.gpsimd.dma_start(out_d.ap()[rows, :], ob[:])

                probs = probs0
                nxt = bridge
                for i in range(NIT):
                    b = i // SQB
                    sq0 = b * S + (i % SQB) * 512
                    # --- ILV: aug matmuls of i interleaved with scores/exp
                    # of i+1 (keeps ScalarE fed while PE runs aug) --------
                    psA = psab.tile([128, 512], f32, tag="pa", name="psA")
                    psB = psab.tile([128, 512], f32, tag="pb", name="psB")
                    for skt in range(SKT):
                        if i + 1 < NIT and skt % 2 == 0 and len(nxt) < SKT:
                            for _ in range(2):
                                j = len(nxt)
                                nxt.append(emit_sc(i + 1, j, j in FAST_SET))
                        g = b * SKT + skt
                        st, sp = (skt == 0), (skt == SKT - 1)
                        nc.tensor.matmul(psA[:], lhsT=aug_sb[:, g, 0:128],
                                         rhs=probs[skt][:, 0:512], start=st, stop=sp,
                                         skip_group_check=True)
                        nc.tensor.matmul(psB[:], lhsT=aug_sb[:, g, 64:192],
                                         rhs=probs[skt][:, 512:1024], start=st, stop=sp,
                                         skip_group_check=True)
                        if skt == 4 and prev is not None:
                            emit_cproj(*prev)
                            prev = None
                    # --- N: normalize --------------------------------------
                    se_st = ripool.tile([128, 512], f32, tag="st", name="se_st")
                    nc.vector.tensor_copy(se_st[64:128, :], psA[64:128, :])
                    nc.vector.tensor_copy(se_st[0:64, :], psB[0:64, :])
                    rec_in = ripool.tile([128, 512], f32, tag="ri", name="rec_in")
                    nc.sync.dma_start(rec_in[0:64, :], se_st[64:128, :])
                    nc.sync.dma_start(rec_in[64:128, :], se_st[0:64, :])
                    rec = rpool.tile([128, 512], f32, tag="rc", name="rec")
                    nc.vector.reciprocal_approx_fast(rec[:], rec_in[:])
                    ctxn = npool.tile([128, 512], bf16, tag="cn", name="ctxn")
                    nc.vector.tensor_tensor(ctxn[0:64, :], psA[0:64, :],
                                            rec[0:64, :], op=MULT)
                    nc.vector.tensor_tensor(ctxn[64:128, :], psB[64:128, :],
                                            rec[64:128, :], op=MULT)
                    # --- C(i-1), if not already emitted mid-ILV ------------
                    if prev is not None:
                        emit_cproj(*prev)
                    prev = (ctxn, sq0)
                    probs = nxt
                    nxt = []

                emit_cproj(*prev, final=True)

    nc.compile()
    return nc


def _enable_ldw_opt():
    """No-op: --enable-ldw-opt=true crashes walrus codegen
    (CoreV3GenImpl visitInstLdweights) on this kernel's LDWEIGHTS mix."""
    return


def _get_nc():
    if "nc" not in _CACHE:
        _enable_ldw_opt()
        _CACHE["nc"] = _build_nc()
    return _CACHE["nc"]


def kernel(hidden_states, attention_mask, c_attn_w, c_attn_b, c_proj_w, c_proj_b):
    from concourse.bass_utils import run_bass_kernel_spmd

    bf16 = ml_dtypes.bfloat16
    hs = np.asarray(hidden_states, dtype=np.float32).reshape(T, H)
    hsT = np.ascontiguousarray(hs.T).astype(bf16)
    mask = np.ascontiguousarray(
        np.broadcast_to(
            np.asarray(attention_mask, dtype=np.float32).reshape(B, 1, 1, S)[:, 0, 0, :],
            (B, S),
        )
    )
    w = np.asarray(c_attn_w, dtype=np.float32)
    bqkv = np.asarray(c_attn_b, dtype=np.float32)
    wp_full = np.asarray(c_proj_w, dtype=np.float32)
    scale = 1.0 / np.sqrt(HD)

    def pack(a):  # [H, DC] -> [128, KC, DC], contiguous per-partition lines
        return np.ascontiguousarray(
            a.reshape(KC, 128, DC).transpose(1, 0, 2)).astype(bf16)

    in_maps = []
    for c in range(NCORES):
        lo, hi = c * DC, (c + 1) * DC
        in_maps.append({
            "hsT": hsT,
            "wq": pack(w[:, lo:hi] * scale),
            "wk": pack(w[:, H + lo:H + hi]),
            "wv": pack(w[:, 2 * H + lo:2 * H + hi]),
            "wp": np.ascontiguousarray(wp_full[lo:hi, :]).astype(bf16),
            "bq": np.ascontiguousarray((bqkv[lo:hi] * scale).reshape(DC, 1)),
            "bk": np.ascontiguousarray(bqkv[H + lo:H + hi].reshape(DC, 1)),
            "mask": mask,
        })

    res = run_bass_kernel_spmd(_get_nc(), in_maps, core_ids=list(range(NCORES)))
    _CACHE["last_result"] = res
    acc = np.zeros((T, H), dtype=np.float32)
    for c in range(NCORES):
        acc += np.asarray(res.results[c]["out"], dtype=np.float32)
    # v-bias contributes the constant row bv @ c_proj_w (exact, host-side)
    bv_full = bqkv[2 * H:3 * H]
    acc += (bv_full @ wp_full + np.asarray(c_proj_b, dtype=np.float32))[None, :]
    return acc.reshape(B, S, H)



# revision 11
# speedup vs baseline: 2.4946x; 2.4946x over previous
"""CodeSage attention (B=2, S=2048, H=1024, 16 heads x 64) on 8 Trainium2 cores.

Sharding: tensor-parallel over heads — 2 heads per core. Each core computes
its head-group's QKV projection, attention, and the c_proj partial product;
the host sums the 8 partials and adds c_proj_b + bv @ c_proj_w (the V-bias
contribution reduces to a constant output row, applied host-side exactly).

Device-side design (bf16 matmuls, fp32 accumulation):

phase 1 (k-outer for stationary reuse, two 4-block passes so the projection
accumulators only occupy 4 PSUM banks and the score pool can coexist):
    qT,kT [128=2*64, T] = Wslice^T @ hsT   (1/sqrt(hd) folded into wq host-side)
    V natural per key tile; the PSUM->aug copy fuses the exp(mask) row scale:
    aug[g] = [ v'_h0(64) | em64(64) | v'_h1(64) ]  where em = exp(mask),
    v' = v*em — this folds the additive mask into V/ones stationaries, making
    the exp bias-free and the sumexp mask-aware.

phase 2, per (batch, 512-query-block) i — software-pipelined and interleaved
so ScalarE (the exp pacer) never starves:
    A: scoresT[sk,sq] = kT-slices^T @ qT  (2-head row-tiled pairs, K=64)
       probs = exp(scoresT)               (ScalarE)
    B: aug matmuls (K=128): psA = [ctx_h0 | se_h0*64], psB = [se_h1*64 | ctx_h1]
    N: DVE copy + DMA realign of sumexp rows, approx-reciprocal, ctxn = ctx*rec
    C: c_proj partial: out_tile = ctxn^T @ wp -> DVE copy -> DMA out
    Emission: Q, K, A(0), V; then for i: ILV[B(i) ~ A(i+1)], N(i), C(i-1).
"""

import numpy as np
import ml_dtypes

B, S, H = 2, 2048, 1024
NH, HD = 16, 64
NCORES = 8
HPC = NH // NCORES          # heads per core = 2
DC = HPC * HD               # per-core head dims = 128
T = B * S                   # 4096 tokens
KC = H // 128               # 8 contraction chunks
NBLK = T // 512             # 8 column blocks of 512 tokens
SQB = S // 512              # 4 query blocks per batch
SKT = S // 128              # 16 key tiles per batch
NIT = B * SQB               # 8 pipelined phase-2 iterations

_CACHE = {}


def _build_nc():
    import concourse.mybir as mybir
    import concourse.tile as tile
    from concourse import bacc

    f32 = mybir.dt.float32
    bf16 = mybir.dt.bfloat16

    nc = bacc.Bacc("TRN2", target_bir_lowering=False, debug=False,
                   num_devices=NCORES)

    hsT_d = nc.dram_tensor("hsT", [H, T], bf16, kind="ExternalInput")
    wq_d = nc.dram_tensor("wq", [128, KC, DC], bf16, kind="ExternalInput")
    wk_d = nc.dram_tensor("wk", [128, KC, DC], bf16, kind="ExternalInput")
    wv_d = nc.dram_tensor("wv", [128, KC, DC], bf16, kind="ExternalInput")
    wp_d = nc.dram_tensor("wp", [DC, H], bf16, kind="ExternalInput")
    bq_d = nc.dram_tensor("bq", [DC, 1], f32, kind="ExternalInput")
    bk_d = nc.dram_tensor("bk", [DC, 1], f32, kind="ExternalInput")
    mask_d = nc.dram_tensor("mask", [B, S], f32, kind="ExternalInput")
    out_d = nc.dram_tensor("out", [T, H], bf16, kind="ExternalOutput")

    i32 = mybir.dt.int32
    i16 = mybir.dt.int16
    EXP = mybir.ActivationFunctionType.Exp
    MULT = mybir.AluOpType.mult
    ADDOP = mybir.AluOpType.add
    # 1-term bf16 Schraudolph exp: pr_bf16 = bitcast_i16(round(s*A16 + B16)).
    # bf16 is the top 16 bits of f32, so the rounded i16 IS the bf16 prob —
    # ONE tensor_scalar per tile (the lone PSUM-evacuation op), no extra
    # engine work. Sawtooth rel err rms 1.8%, mean-free; on 3/16 key tiles
    # the ctx perturbation is ~0.9%. Offloads exp from ScalarE (the phase-2
    # pacer, saturated at ~17.2us/iter) onto DVE.
    SCH_A16 = 12102203.161561485 / 65536.0   # 2^7 * log2(e)
    SCH_B16 = 16248.576                      # 2^7 * (127 - 0.058)
    FAST_SET = (2, 7, 12)               # ILV tile indices on the fast path

    with tile.TileContext(nc) as tc:
        with (
            tc.tile_pool(name="const", bufs=1) as cpool,
            tc.tile_pool(name="qkv", bufs=1) as qpool,
            tc.tile_pool(name="probs", bufs=27) as ppool,
            tc.tile_pool(name="ctxn", bufs=2) as npool,
            tc.tile_pool(name="rcin", bufs=2) as ripool,
            tc.tile_pool(name="rec", bufs=2) as rpool,
            tc.tile_pool(name="ob", bufs=3) as opool,
            tc.tile_pool(name="ps_sc", bufs=2, space="PSUM") as pssc,
        ):
            wq_sb = cpool.tile([128, KC, DC], bf16)
            wk_sb = cpool.tile([128, KC, DC], bf16)
            wv_sb = cpool.tile([128, KC, DC], bf16)
            wp_sb = cpool.tile([DC, H], bf16)
            bq_sb = cpool.tile([DC, 1], f32)
            bk_sb = cpool.tile([DC, 1], f32)
            ones64 = cpool.tile([128, 64], bf16)
            mask_sb = cpool.tile([128, B, SKT], f32)
            em_sb = cpool.tile([128, B, SKT], f32)

            # ramp-critical DMAs only: wk+wq gate the first proj wave;
            # everything else is issued after the hs quarter-0 transfers.
            nc.sync.dma_start(wk_sb[:], wk_d.ap())
            nc.sync.dma_start(wq_sb[:], wq_d.ap())
            nc.vector.memset(ones64[:], 1.0)

            qT_sb = qpool.tile([128, T], bf16)   # rows 0:64 head0, 64:128 head1
            kT_sb = qpool.tile([128, T], bf16)
            # aug stationaries: per key tile g, [v'_h0 | em64 | v'_h1]
            aug_sb = qpool.tile([128, B * SKT, 192], bf16)

            # ---- phase-2 emission helpers ---------------------------------
            def emit_sc(i, skt, fast=False):
                b = i // SQB
                sq0 = b * S + (i % SQB) * 512
                sk = slice(b * S + skt * 128, b * S + skt * 128 + 128)
                sq = slice(sq0, sq0 + 512)
                sc_ps = pssc.tile([128, 1024], f32, tag="sc", name="sc_ps")
                nc.tensor.matmul(sc_ps[:, 0:512], lhsT=kT_sb[0:64, sk],
                                 rhs=qT_sb[0:64, sq], start=True, stop=True,
                                 skip_group_check=True)
                nc.tensor.matmul(sc_ps[:, 512:1024], lhsT=kT_sb[64:128, sk],
                                 rhs=qT_sb[64:128, sq], start=True, stop=True,
                                 skip_group_check=True)
                pr = ppool.tile([128, 1024], bf16, tag="pr", name="pr")
                if fast:
                    nc.vector.tensor_scalar(pr[:].bitcast(i16), sc_ps[:],
                                            SCH_A16, SCH_B16,
                                            op0=MULT, op1=ADDOP)
                else:
                    nc.scalar.activation(pr[:], sc_ps[:], EXP)
                return pr

            # ---- phase 1: QKV projection ----------------------------------
            with (
                tc.tile_pool(name="hs", bufs=1) as hpool,
                tc.tile_pool(name="ps1", bufs=1, space="PSUM") as ps1,
            ):
                hs_all = hpool.tile([128, KC, T], bf16)
                # quarter-chunk transfers, quarter-major: the first proj pass
                # (kT/qT blocks 0-1) only reads column-quarter 0 of every
                # chunk, so emitting all chunks' quarter-0 first lets scores
                # start after ~2 MB of input instead of all 8 MB.
                for q4 in range(4):
                    cs = slice(q4 * 1024, (q4 + 1) * 1024)
                    for k in range(KC):
                        # alternate issue queues: halves the serial DMA-issue
                        # ramp (sync also carries wk/wq ahead of chunk k=0)
                        eng = nc.sync if k % 2 == 0 else nc.gpsimd
                        eng.dma_start(hs_all[:, k, cs],
                                      hsT_d.ap()[k * 128:(k + 1) * 128, cs])
                    if q4 == 0:
                        # deferred non-ramp-critical inputs, by first use:
                        # biases at the first proj writes, mask/em at the V
                        # phase, wp only at cproj
                        nc.sync.dma_start(bq_sb[:], bq_d.ap())
                        nc.sync.dma_start(bk_sb[:], bk_d.ap())
                        nc.sync.dma_start(
                            mask_sb[:],
                            mask_d.ap().rearrange("b (t p) -> p b t", p=128))
                        nc.scalar.activation(em_sb[:], mask_sb[:], EXP)
                    elif q4 == 1:
                        nc.gpsimd.dma_start(wv_sb[:], wv_d.ap())
                    elif q4 == 2:
                        nc.gpsimd.dma_start(wp_sb[:], wp_d.ap())

                def proj_joint(lanes):
                    """k-major accumulation over several (w, dst, bias, blk)
                    lanes at once — paces with arriving hs chunks."""
                    ps = [ps1.tile([128, 512], f32, tag=f"b{j}", name=f"ps_j{j}")
                          for j in range(len(lanes))]
                    for k in range(KC):
                        for j, (w_sb, _, _, blk) in enumerate(lanes):
                            cols = slice(blk * 512, (blk + 1) * 512)
                            nc.tensor.matmul(ps[j][:], lhsT=w_sb[:, k, :],
                                             rhs=hs_all[:, k, cols],
                                             start=(k == 0), stop=(k == KC - 1),
                                             skip_group_check=True)
                    for j, (_, dst_sb, bias_sb, blk) in enumerate(lanes):
                        cols = slice(blk * 512, (blk + 1) * 512)
                        nc.vector.tensor_scalar_add(dst_sb[:, cols], ps[j][:],
                                                    bias_sb[:, 0:1])

                KL = (wk_sb, kT_sb, bk_sb)
                QL = (wq_sb, qT_sb, bq_sb)
                # batch-0 K and Q first; exp for iteration 0 starts as soon as
                # the needed kT/qT halves exist, while PE continues projecting.
                proj_joint([KL + (0,), KL + (1,), QL + (0,)])
                probs0 = [emit_sc(0, skt) for skt in range(SKT // 2)]
                proj_joint([KL + (2,), KL + (3,), QL + (1,)])
                probs0 += [emit_sc(0, skt) for skt in range(SKT // 2, SKT)]
                proj_joint([QL + (2,), QL + (3,)])
                proj_joint([KL + (4,), KL + (5,), QL + (4,), QL + (5,)])
                proj_joint([KL + (6,), KL + (7,), QL + (6,), QL + (7,)])

                # V natural per key tile g; PSUM->aug copy fuses em scaling.
                # Bridge pairs of iteration-1 scores/exp into the V loop so
                # ScalarE doesn't starve between A(0) and the first ILV.
                # em columns for all 32 tiles in one broadcast op (em is
                # per-(partition, tile); replicate across the 64 cols)
                em_bc = em_sb[:].rearrange("p b t -> p (b t)").unsqueeze(2)
                nc.vector.tensor_scalar_mul(
                    aug_sb[:, :, 64:128], em_bc.broadcast_to([128, B * SKT, 64]),
                    1.0)
                bridge = []
                for g in range(B * SKT):
                    b, skt = g // SKT, g % SKT
                    em = em_sb[:, b, skt:skt + 1]
                    gc = slice(g * 128, (g + 1) * 128)
                    v_ps = ps1.tile([128, DC], f32, tag=f"b{g % 4}", name="v_ps")
                    for k in range(KC):
                        nc.tensor.matmul(v_ps[:], lhsT=hs_all[:, k, gc],
                                         rhs=wv_sb[:, k, :],
                                         start=(k == 0), stop=(k == KC - 1),
                                         skip_group_check=True)
                    # both 64-wide v halves in one op: out cols {0:64,128:192}
                    aug_v = aug_sb[:, g, :].rearrange("p (a b) -> p a b", a=3)[:, 0:3:2, :]
                    src_v = v_ps[:].rearrange("p (two c) -> p two c", two=2)
                    nc.vector.tensor_scalar_mul(aug_v, src_v, em)
                    if NIT > 1 and g % 3 == 2 and len(bridge) < 10:
                        bridge.append(emit_sc(1, len(bridge)))
                        bridge.append(emit_sc(1, len(bridge)))

            # ---- phase 2: attention + c_proj ------------------------------
            with tc.tile_pool(name="ps_ab", bufs=2, space="PSUM") as psab:
                prev = None  # (ctxn tile, sq0) pending c_proj

                def emit_cproj(ctxn, sq0, final=False):
                    # output DMA rides the idle GpSimd queue so it cannot
                    # delay the sync-queue rec_in realign DMAs (normalize
                    # critical path)
                    for t4 in range(4):
                        tok = slice(t4 * 128, (t4 + 1) * 128)
                        rows = slice(sq0 + t4 * 128, sq0 + (t4 + 1) * 128)
                        op_a = psab.tile([128, 512], f32, tag="pa", name="op_a")
                        op_b = psab.tile([128, 512], f32, tag="pb", name="op_b")
                        nc.tensor.matmul(op_a[:], lhsT=ctxn[:, tok],
                                         rhs=wp_sb[:, 0:512], start=True, stop=True,
                                         skip_group_check=True)
                        nc.tensor.matmul(op_b[:], lhsT=ctxn[:, tok],
                                         rhs=wp_sb[:, 512:1024], start=True, stop=True,
                                         skip_group_check=True)
                        ob = opool.tile([128, 1024], bf16, tag="ob", name="ob")
                        nc.vector.tensor_copy(ob[:, 0:512], op_a[:])
                        if final:
                            # exp stream is over: ScalarE is idle, split the
                            # PSUM evacuation so the tail isn't DVE-serial
                            nc.scalar.copy(ob[:, 512:1024], op_b[:])
                        else:
                            nc.vector.tensor_copy(ob[:, 512:1024], op_b[:])
                        nc.gpsimd.dma_start(out_d.ap()[rows, :], ob[:])

                probs = probs0
                nxt = bridge
                for i in range(NIT):
                    b = i // SQB
                    sq0 = b * S + (i % SQB) * 512
                    # --- ILV: aug matmuls of i interleaved with scores/exp
                    # of i+1 (keeps ScalarE fed while PE runs aug) --------
                    psA = psab.tile([128, 512], f32, tag="pa", name="psA")
                    psB = psab.tile([128, 512], f32, tag="pb", name="psB")
                    for skt in range(SKT):
                        if i + 1 < NIT and skt % 2 == 0 and len(nxt) < SKT:
                            for _ in range(2):
                                j = len(nxt)
                                nxt.append(emit_sc(i + 1, j, j in FAST_SET))
                        g = b * SKT + skt
                        st, sp = (skt == 0), (skt == SKT - 1)
                        nc.tensor.matmul(psA[:], lhsT=aug_sb[:, g, 0:128],
                                         rhs=probs[skt][:, 0:512], start=st, stop=sp,
                                         skip_group_check=True)
                        nc.tensor.matmul(psB[:], lhsT=aug_sb[:, g, 64:192],
                                         rhs=probs[skt][:, 512:1024], start=st, stop=sp,
                                         skip_group_check=True)
                        if skt == 4 and prev is not None:
                            emit_cproj(*prev)
                            prev = None
                    # --- N: normalize --------------------------------------
                    se_st = ripool.tile([128, 512], f32, tag="st", name="se_st")
                    nc.vector.tensor_copy(se_st[64:128, :], psA[64:128, :])
                    nc.vector.tensor_copy(se_st[0:64, :], psB[0:64, :])
                    rec_in = ripool.tile([128, 512], f32, tag="ri", name="rec_in")
                    nc.sync.dma_start(rec_in[0:64, :], se_st[64:128, :])
                    nc.sync.dma_start(rec_in[64:128, :], se_st[0:64, :])
                    rec = rpool.tile([128, 512], f32, tag="rc", name="rec")
                    nc.vector.reciprocal_approx_fast(rec[:], rec_in[:])
                    ctxn = npool.tile([128, 512], bf16, tag="cn", name="ctxn")
                    nc.vector.tensor_tensor(ctxn[0:64, :], psA[0:64, :],
                                            rec[0:64, :], op=MULT)
                    nc.vector.tensor_tensor(ctxn[64:128, :], psB[64:128, :],
                                            rec[64:128, :], op=MULT)
                    # --- C(i-1), if not already emitted mid-ILV ------------
                    if prev is not None:
                        emit_cproj(*prev)
                    prev = (ctxn, sq0)
                    probs = nxt
                    nxt = []

                emit_cproj(*prev, final=True)

    nc.compile()
    return nc


def _enable_ldw_opt():
    """No-op: --enable-ldw-opt=true crashes walrus codegen
    (CoreV3GenImpl visitInstLdweights) on this kernel's LDWEIGHTS mix."""
    return


def _get_nc():
    if "nc" not in _CACHE:
        _enable_ldw_opt()
        _CACHE["nc"] = _build_nc()
    return _CACHE["nc"]


def kernel(hidden_states, attention_mask, c_attn_w, c_attn_b, c_proj_w, c_proj_b):
    from concourse.bass_utils import run_bass_kernel_spmd

    bf16 = ml_dtypes.bfloat16
    hs = np.asarray(hidden_states, dtype=np.float32).reshape(T, H)
    hsT = np.ascontiguousarray(hs.T).astype(bf16)
    mask = np.ascontiguousarray(
        np.broadcast_to(
            np.asarray(attention_mask, dtype=np.float32).reshape(B, 1, 1, S)[:, 0, 0, :],
            (B, S),
        )
    )
    w = np.asarray(c_attn_w, dtype=np.float32)
    bqkv = np.asarray(c_attn_b, dtype=np.float32)
    wp_full = np.asarray(c_proj_w, dtype=np.float32)
    scale = 1.0 / np.sqrt(HD)

    def pack(a):  # [H, DC] -> [128, KC, DC], contiguous per-partition lines
        return np.ascontiguousarray(
            a.reshape(KC, 128, DC).transpose(1, 0, 2)).astype(bf16)

    in_maps = []
    for c in range(NCORES):
        lo, hi = c * DC, (c + 1) * DC
        in_maps.append({
            "hsT": hsT,
            "wq": pack(w[:, lo:hi] * scale),
            "wk": pack(w[:, H + lo:H + hi]),
            "wv": pack(w[:, 2 * H + lo:2 * H + hi]),
            "wp": np.ascontiguousarray(wp_full[lo:hi, :]).astype(bf16),
            "bq": np.ascontiguousarray((bqkv[lo:hi] * scale).reshape(DC, 1)),
            "bk": np.ascontiguousarray(bqkv[H + lo:H + hi].reshape(DC, 1)),
            "mask": mask,
        })

    res = run_bass_kernel_spmd(_get_nc(), in_maps, core_ids=list(range(NCORES)))
    _CACHE["last_result"] = res
    acc = np.zeros((T, H), dtype=np.float32)
    for c in range(NCORES):
        acc += np.asarray(res.results[c]["out"], dtype=np.float32)
    # v-bias contributes the constant row bv @ c_proj_w (exact, host-side)
    bv_full = bqkv[2 * H:3 * H]
    acc += (bv_full @ wp_full + np.asarray(c_proj_b, dtype=np.float32))[None, :]
    return acc.reshape(B, S, H)



# revision 21
# speedup vs baseline: 2.5965x; 1.0409x over previous
"""CodeSage attention (B=2, S=2048, H=1024, 16 heads x 64) on 8 Trainium2 cores.

Sharding: tensor-parallel over heads — 2 heads per core. Each core computes
its head-group's QKV projection, attention, and the c_proj partial product;
the host sums the 8 partials and adds c_proj_b + bv @ c_proj_w (the V-bias
contribution reduces to a constant output row, applied host-side exactly).

Device-side design (bf16 matmuls, fp32 accumulation):

phase 1 (k-outer for stationary reuse, two 4-block passes so the projection
accumulators only occupy 4 PSUM banks and the score pool can coexist):
    qT,kT [128=2*64, T] = Wslice^T @ hsT   (1/sqrt(hd) folded into wq host-side)
    V natural per key tile; the PSUM->aug copy fuses the exp(mask) row scale:
    aug[g] = [ v'_h0(64) | em64(64) | v'_h1(64) ]  where em = exp(mask),
    v' = v*em — this folds the additive mask into V/ones stationaries, making
    the exp bias-free and the sumexp mask-aware.

phase 2, per (batch, 512-query-block) i — software-pipelined and interleaved
so ScalarE (the exp pacer) never starves:
    A: scoresT[sk,sq] = kT-slices^T @ qT  (2-head row-tiled pairs, K=64)
       probs = exp(scoresT)               (ScalarE)
    B: aug matmuls (K=128): psA = [ctx_h0 | se_h0*64], psB = [se_h1*64 | ctx_h1]
    N: DVE copy + DMA realign of sumexp rows, approx-reciprocal, ctxn = ctx*rec
    C: c_proj partial: out_tile = ctxn^T @ wp -> DVE copy -> DMA out
    Emission: Q, K, A(0), V; then for i: ILV[B(i) ~ A(i+1)], N(i), C(i-1).
"""

import numpy as np
import ml_dtypes
from collections import deque

B, S, H = 2, 2048, 1024
NH, HD = 16, 64
NCORES = 8
HPC = NH // NCORES          # heads per core = 2
DC = HPC * HD               # per-core head dims = 128
T = B * S                   # 4096 tokens
KC = H // 128               # 8 contraction chunks
NBLK = T // 512             # 8 column blocks of 512 tokens
SQB = S // 512              # 4 query blocks per batch
SKT = S // 128              # 16 key tiles per batch
NIT = B * SQB               # 8 pipelined phase-2 iterations

_CACHE = {}


def _build_nc():
    import concourse.mybir as mybir
    import concourse.tile as tile
    from concourse import bacc

    f32 = mybir.dt.float32
    bf16 = mybir.dt.bfloat16

    nc = bacc.Bacc("TRN2", target_bir_lowering=False, debug=False,
                   num_devices=NCORES)

    hsT_d = nc.dram_tensor("hsT", [H, T], bf16, kind="ExternalInput")
    wq_d = nc.dram_tensor("wq", [128, KC, DC], bf16, kind="ExternalInput")
    wk_d = nc.dram_tensor("wk", [128, KC, DC], bf16, kind="ExternalInput")
    wv_d = nc.dram_tensor("wv", [128, KC, DC], bf16, kind="ExternalInput")
    wp_d = nc.dram_tensor("wp", [DC, H], bf16, kind="ExternalInput")
    bq_d = nc.dram_tensor("bq", [DC, 1], f32, kind="ExternalInput")
    bk_d = nc.dram_tensor("bk", [DC, 1], f32, kind="ExternalInput")
    mask_d = nc.dram_tensor("mask", [B, S], f32, kind="ExternalInput")
    out_d = nc.dram_tensor("out", [T, H], bf16, kind="ExternalOutput")

    i32 = mybir.dt.int32
    i16 = mybir.dt.int16
    EXP = mybir.ActivationFunctionType.Exp
    MULT = mybir.AluOpType.mult
    ADDOP = mybir.AluOpType.add
    # 1-term bf16 Schraudolph exp: pr_bf16 = bitcast_i16(round(s*A16 + B16)).
    # bf16 is the top 16 bits of f32, so the rounded i16 IS the bf16 prob —
    # ONE tensor_scalar per tile (the lone PSUM-evacuation op), no extra
    # engine work. Sawtooth rel err rms 1.8%, mean-free; on 3/16 key tiles
    # the ctx perturbation is ~0.9%. Offloads exp from ScalarE (the phase-2
    # pacer, saturated at ~17.2us/iter) onto DVE.
    SCH_A16 = 12102203.161561485 / 65536.0   # 2^7 * log2(e)
    SCH_B16 = 16248.576                      # 2^7 * (127 - 0.058)
    FAST_SET = (2, 7, 12)               # ILV tile indices on the fast path

    with tile.TileContext(nc) as tc:
        with (
            tc.tile_pool(name="const", bufs=1) as cpool,
            tc.tile_pool(name="qkv", bufs=1) as qpool,
            tc.tile_pool(name="probs", bufs=43) as ppool,
            tc.tile_pool(name="ctxn", bufs=2) as npool,
            tc.tile_pool(name="rcin", bufs=2) as ripool,
            tc.tile_pool(name="rec", bufs=2) as rpool,
            tc.tile_pool(name="ob", bufs=3) as opool,
            tc.tile_pool(name="ps_sc", bufs=2, space="PSUM") as pssc,
        ):
            wq_sb = cpool.tile([128, KC, DC], bf16)
            wk_sb = cpool.tile([128, KC, DC], bf16)
            wv_sb = cpool.tile([128, KC, DC], bf16)
            wp_sb = cpool.tile([DC, H], bf16)
            bq_sb = cpool.tile([DC, 1], f32)
            bk_sb = cpool.tile([DC, 1], f32)
            ones64 = cpool.tile([128, 64], bf16)
            mask_sb = cpool.tile([128, B, SKT], f32)
            em_sb = cpool.tile([128, B, SKT], f32)

            # ramp-critical DMAs only: wk+wq gate the first proj wave;
            # everything else is issued after the hs quarter-0 transfers.
            nc.sync.dma_start(wk_sb[:], wk_d.ap())
            nc.sync.dma_start(wq_sb[:], wq_d.ap())
            nc.vector.memset(ones64[:], 1.0)

            qT_sb = qpool.tile([128, T], bf16)   # rows 0:64 head0, 64:128 head1
            kT_sb = qpool.tile([128, T], bf16)
            # aug stationaries: per key tile g, [v'_h0 | em64 | v'_h1]
            aug_sb = qpool.tile([128, B * SKT, 192], bf16)

            # ---- phase-2 emission helpers ---------------------------------
            def emit_sc(i, skt, fast=False):
                b = i // SQB
                sq0 = b * S + (i % SQB) * 512
                sk = slice(b * S + skt * 128, b * S + skt * 128 + 128)
                sq = slice(sq0, sq0 + 512)
                sc_ps = pssc.tile([128, 1024], f32, tag="sc", name="sc_ps")
                nc.tensor.matmul(sc_ps[:, 0:512], lhsT=kT_sb[0:64, sk],
                                 rhs=qT_sb[0:64, sq], start=True, stop=True,
                                 skip_group_check=True)
                nc.tensor.matmul(sc_ps[:, 512:1024], lhsT=kT_sb[64:128, sk],
                                 rhs=qT_sb[64:128, sq], start=True, stop=True,
                                 skip_group_check=True)
                pr = ppool.tile([128, 1024], bf16, tag="pr", name="pr")
                if fast:
                    nc.vector.tensor_scalar(pr[:].bitcast(i16), sc_ps[:],
                                            SCH_A16, SCH_B16,
                                            op0=MULT, op1=ADDOP)
                else:
                    nc.scalar.activation(pr[:], sc_ps[:], EXP)
                return pr

            # ---- phase 1: QKV projection ----------------------------------
            with (
                tc.tile_pool(name="hs", bufs=1) as hpool,
                tc.tile_pool(name="ps1", bufs=1, space="PSUM") as ps1,
            ):
                hs_all = hpool.tile([128, KC, T], bf16)
                # quarter-chunk transfers, quarter-major: the first proj pass
                # (kT/qT blocks 0-1) only reads column-quarter 0 of every
                # chunk, so emitting all chunks' quarter-0 first lets scores
                # start after ~2 MB of input instead of all 8 MB.
                for q4 in range(4):
                    cs = slice(q4 * 1024, (q4 + 1) * 1024)
                    for k in range(KC):
                        nc.sync.dma_start(hs_all[:, k, cs],
                                          hsT_d.ap()[k * 128:(k + 1) * 128, cs])
                    if q4 == 0:
                        # deferred non-ramp-critical inputs, by first use:
                        # biases at the first proj writes, mask/em at the V
                        # phase, wp only at cproj
                        nc.sync.dma_start(bq_sb[:], bq_d.ap())
                        nc.sync.dma_start(bk_sb[:], bk_d.ap())
                        nc.sync.dma_start(
                            mask_sb[:],
                            mask_d.ap().rearrange("b (t p) -> p b t", p=128))
                        nc.scalar.activation(em_sb[:], mask_sb[:], EXP)
                    elif q4 == 1:
                        nc.sync.dma_start(wv_sb[:], wv_d.ap())
                    elif q4 == 2:
                        nc.sync.dma_start(wp_sb[:], wp_d.ap())

                def proj_joint(lanes):
                    """k-major accumulation over several (w, dst, bias, blk)
                    lanes at once — paces with arriving hs chunks."""
                    ps = [ps1.tile([128, 512], f32, tag=f"b{j}", name=f"ps_j{j}")
                          for j in range(len(lanes))]
                    for k in range(KC):
                        for j, (w_sb, _, _, blk) in enumerate(lanes):
                            cols = slice(blk * 512, (blk + 1) * 512)
                            nc.tensor.matmul(ps[j][:], lhsT=w_sb[:, k, :],
                                             rhs=hs_all[:, k, cols],
                                             start=(k == 0), stop=(k == KC - 1),
                                             skip_group_check=True)
                    for j, (_, dst_sb, bias_sb, blk) in enumerate(lanes):
                        cols = slice(blk * 512, (blk + 1) * 512)
                        nc.vector.tensor_scalar_add(dst_sb[:, cols], ps[j][:],
                                                    bias_sb[:, 0:1])

                KL = (wk_sb, kT_sb, bk_sb)
                QL = (wq_sb, qT_sb, bq_sb)
                # batch-0 K and Q first; exp for iteration 0 starts as soon as
                # the needed kT/qT halves exist, while PE continues projecting.
                proj_joint([KL + (0,), KL + (1,), QL + (0,)])
                probs0 = [emit_sc(0, skt) for skt in range(SKT // 2)]
                proj_joint([KL + (2,), KL + (3,), QL + (1,)])
                probs0 += [emit_sc(0, skt) for skt in range(SKT // 2, SKT)]
                proj_joint([QL + (2,), QL + (3,)])
                proj_joint([KL + (4,), KL + (5,), QL + (4,), QL + (5,)])
                proj_joint([KL + (6,), KL + (7,), QL + (6,), QL + (7,)])

                # ---- lookahead score emission cursor ----------------------
                # scores/exp for iters 1..NIT-1 are emitted ahead of their
                # consuming ILV (probs queue holds up to 2 iterations) so the
                # exp stream on ScalarE decouples from per-iteration pacing:
                # ACT's surplus capacity in one window pre-computes the next.
                probs_q = deque()
                cur = [1, 0]

                def emit_next(budget):
                    while budget > 0 and cur[0] < NIT and len(probs_q) < 25:
                        i, j = cur
                        fast = j in FAST_SET and i in (1, 2)
                        probs_q.append(emit_sc(i, j, fast))
                        cur[:] = (i, j + 1) if j + 1 < SKT else (i + 1, 0)
                        budget -= 1

                # V natural per key tile g; PSUM->aug copy fuses em scaling.
                # Interleave lookahead scores/exp into the V loop (every other
                # g: each emitted score window inserts a PE score-pair that
                # must wait on the exp stream, so denser emission would make
                # the V pass ACT-paced).
                # em columns for all 32 tiles in one broadcast op (em is
                # per-(partition, tile); replicate across the 64 cols)
                em_bc = em_sb[:].rearrange("p b t -> p (b t)").unsqueeze(2)
                nc.vector.tensor_scalar_mul(
                    aug_sb[:, :, 64:128], em_bc.broadcast_to([128, B * SKT, 64]),
                    1.0)
                for g in range(B * SKT):
                    b, skt = g // SKT, g % SKT
                    em = em_sb[:, b, skt:skt + 1]
                    gc = slice(g * 128, (g + 1) * 128)
                    v_ps = ps1.tile([128, DC], f32, tag=f"b{g % 4}", name="v_ps")
                    for k in range(KC):
                        nc.tensor.matmul(v_ps[:], lhsT=hs_all[:, k, gc],
                                         rhs=wv_sb[:, k, :],
                                         start=(k == 0), stop=(k == KC - 1),
                                         skip_group_check=True)
                    # both 64-wide v halves in one op: out cols {0:64,128:192}
                    aug_v = aug_sb[:, g, :].rearrange("p (a b) -> p a b", a=3)[:, 0:3:2, :]
                    src_v = v_ps[:].rearrange("p (two c) -> p two c", two=2)
                    nc.vector.tensor_scalar_mul(aug_v, src_v, em)
                    if g % 2 == 1:
                        emit_next(1)

            # ---- phase 2: attention + c_proj ------------------------------
            with tc.tile_pool(name="ps_ab", bufs=2, space="PSUM") as psab:
                prev = None  # (ctxn tile, sq0) pending c_proj

                def emit_cproj(ctxn, sq0, final=False):
                    # output DMA rides the idle GpSimd queue so it cannot
                    # delay the sync-queue rec_in realign DMAs (normalize
                    # critical path)
                    for t4 in range(4):
                        tok = slice(t4 * 128, (t4 + 1) * 128)
                        rows = slice(sq0 + t4 * 128, sq0 + (t4 + 1) * 128)
                        op_a = psab.tile([128, 512], f32, tag="pa", name="op_a")
                        op_b = psab.tile([128, 512], f32, tag="pb", name="op_b")
                        nc.tensor.matmul(op_a[:], lhsT=ctxn[:, tok],
                                         rhs=wp_sb[:, 0:512], start=True, stop=True,
                                         skip_group_check=True)
                        nc.tensor.matmul(op_b[:], lhsT=ctxn[:, tok],
                                         rhs=wp_sb[:, 512:1024], start=True, stop=True,
                                         skip_group_check=True)
                        ob = opool.tile([128, 1024], bf16, tag="ob", name="ob")
                        nc.vector.tensor_copy(ob[:, 0:512], op_a[:])
                        if final:
                            # exp stream is over: ScalarE is idle, split the
                            # PSUM evacuation so the tail isn't DVE-serial
                            nc.scalar.copy(ob[:, 512:1024], op_b[:])
                        else:
                            nc.vector.tensor_copy(ob[:, 512:1024], op_b[:])
                        nc.gpsimd.dma_start(out_d.ap()[rows, :], ob[:])

                probs = probs0
                for i in range(NIT):
                    b = i // SQB
                    sq0 = b * S + (i % SQB) * 512
                    # --- ILV: aug matmuls of i interleaved with lookahead
                    # scores/exp emission (keeps ScalarE fed while PE runs
                    # aug; the probs queue absorbs exp-latency spikes) ----
                    psA = psab.tile([128, 512], f32, tag="pa", name="psA")
                    psB = psab.tile([128, 512], f32, tag="pb", name="psB")
                    for skt in range(SKT):
                        if skt % 2 == 0:
                            emit_next(2)
                        g = b * SKT + skt
                        st, sp = (skt == 0), (skt == SKT - 1)
                        nc.tensor.matmul(psA[:], lhsT=aug_sb[:, g, 0:128],
                                         rhs=probs[skt][:, 0:512], start=st, stop=sp,
                                         skip_group_check=True)
                        nc.tensor.matmul(psB[:], lhsT=aug_sb[:, g, 64:192],
                                         rhs=probs[skt][:, 512:1024], start=st, stop=sp,
                                         skip_group_check=True)
                        if skt == 4 and prev is not None:
                            emit_cproj(*prev)
                            prev = None
                    # --- N: normalize --------------------------------------
                    se_st = ripool.tile([128, 512], f32, tag="st", name="se_st")
                    nc.vector.tensor_copy(se_st[64:128, :], psA[64:128, :])
                    nc.vector.tensor_copy(se_st[0:64, :], psB[0:64, :])
                    rec_in = ripool.tile([128, 512], f32, tag="ri", name="rec_in")
                    nc.sync.dma_start(rec_in[0:64, :], se_st[64:128, :])
                    nc.sync.dma_start(rec_in[64:128, :], se_st[0:64, :])
                    rec = rpool.tile([128, 512], f32, tag="rc", name="rec")
                    nc.vector.reciprocal_approx_fast(rec[:], rec_in[:])
                    ctxn = npool.tile([128, 512], bf16, tag="cn", name="ctxn")
                    nc.vector.tensor_tensor(ctxn[0:64, :], psA[0:64, :],
                                            rec[0:64, :], op=MULT)
                    nc.vector.tensor_tensor(ctxn[64:128, :], psB[64:128, :],
                                            rec[64:128, :], op=MULT)
                    # --- C(i-1), if not already emitted mid-ILV ------------
                    if prev is not None:
                        emit_cproj(*prev)
                    prev = (ctxn, sq0)
                    if i + 1 < NIT:
                        emit_next(SKT)  # top up if the queue ran dry
                        probs = [probs_q.popleft() for _ in range(SKT)]

                emit_cproj(*prev, final=True)

    nc.compile()
    return nc


def _enable_ldw_opt():
    """No-op: --enable-ldw-opt=true crashes walrus codegen
    (CoreV3GenImpl visitInstLdweights) on this kernel's LDWEIGHTS mix."""
    return


def _get_nc():
    if "nc" not in _CACHE:
        _enable_ldw_opt()
        _CACHE["nc"] = _build_nc()
    return _CACHE["nc"]


def kernel(hidden_states, attention_mask, c_attn_w, c_attn_b, c_proj_w, c_proj_b):
    from concourse.bass_utils import run_bass_kernel_spmd

    bf16 = ml_dtypes.bfloat16
    hs = np.asarray(hidden_states, dtype=np.float32).reshape(T, H)
    hsT = np.ascontiguousarray(hs.T).astype(bf16)
    mask = np.ascontiguousarray(
        np.broadcast_to(
            np.asarray(attention_mask, dtype=np.float32).reshape(B, 1, 1, S)[:, 0, 0, :],
            (B, S),
        )
    )
    w = np.asarray(c_attn_w, dtype=np.float32)
    bqkv = np.asarray(c_attn_b, dtype=np.float32)
    wp_full = np.asarray(c_proj_w, dtype=np.float32)
    scale = 1.0 / np.sqrt(HD)

    def pack(a):  # [H, DC] -> [128, KC, DC], contiguous per-partition lines
        return np.ascontiguousarray(
            a.reshape(KC, 128, DC).transpose(1, 0, 2)).astype(bf16)

    in_maps = []
    for c in range(NCORES):
        lo, hi = c * DC, (c + 1) * DC
        in_maps.append({
            "hsT": hsT,
            "wq": pack(w[:, lo:hi] * scale),
            "wk": pack(w[:, H + lo:H + hi]),
            "wv": pack(w[:, 2 * H + lo:2 * H + hi]),
            "wp": np.ascontiguousarray(wp_full[lo:hi, :]).astype(bf16),
            "bq": np.ascontiguousarray((bqkv[lo:hi] * scale).reshape(DC, 1)),
            "bk": np.ascontiguousarray(bqkv[H + lo:H + hi].reshape(DC, 1)),
            "mask": mask,
        })

    res = run_bass_kernel_spmd(_get_nc(), in_maps, core_ids=list(range(NCORES)))
    _CACHE["last_result"] = res
    acc = np.zeros((T, H), dtype=np.float32)
    for c in range(NCORES):
        acc += np.asarray(res.results[c]["out"], dtype=np.float32)
    # v-bias contributes the constant row bv @ c_proj_w (exact, host-side)
    bv_full = bqkv[2 * H:3 * H]
    acc += (bv_full @ wp_full + np.asarray(c_proj_b, dtype=np.float32))[None, :]
    return acc.reshape(B, S, H)



# revision 26
# speedup vs baseline: 2.6476x; 1.0197x over previous
"""CodeSage attention (B=2, S=2048, H=1024, 16 heads x 64) on 8 Trainium2 cores.

Sharding: tensor-parallel over heads — 2 heads per core. Each core computes
its head-group's QKV projection, attention, and the c_proj partial product;
the host sums the 8 partials and adds c_proj_b + bv @ c_proj_w (the V-bias
contribution reduces to a constant output row, applied host-side exactly).

Device-side design (bf16 matmuls, fp32 accumulation):

phase 1 (k-outer for stationary reuse, two 4-block passes so the projection
accumulators only occupy 4 PSUM banks and the score pool can coexist):
    qT,kT [128=2*64, T] = Wslice^T @ hsT   (1/sqrt(hd) folded into wq host-side)
    V natural per key tile; the PSUM->aug copy fuses the exp(mask) row scale:
    aug[g] = [ v'_h0(64) | em64(64) | v'_h1(64) ]  where em = exp(mask),
    v' = v*em — this folds the additive mask into V/ones stationaries, making
    the exp bias-free and the sumexp mask-aware.

phase 2, per (batch, 512-query-block) i — software-pipelined and interleaved
so ScalarE (the exp pacer) never starves:
    A: scoresT[sk,sq] = kT-slices^T @ qT  (2-head row-tiled pairs, K=64)
       probs = exp(scoresT)               (ScalarE)
    B: aug matmuls (K=128): psA = [ctx_h0 | se_h0*64], psB = [se_h1*64 | ctx_h1]
    N: DVE copy + DMA realign of sumexp rows, approx-reciprocal, ctxn = ctx*rec
    C: c_proj partial: out_tile = ctxn^T @ wp -> DVE copy -> DMA out
    Emission: Q, K, A(0), V; then for i: ILV[B(i) ~ A(i+1)], N(i), C(i-1).
"""

import numpy as np
import ml_dtypes
from collections import deque

B, S, H = 2, 2048, 1024
NH, HD = 16, 64
NCORES = 8
HPC = NH // NCORES          # heads per core = 2
DC = HPC * HD               # per-core head dims = 128
T = B * S                   # 4096 tokens
KC = H // 128               # 8 contraction chunks
NBLK = T // 512             # 8 column blocks of 512 tokens
SQB = S // 512              # 4 query blocks per batch
SKT = S // 128              # 16 key tiles per batch
NIT = B * SQB               # 8 pipelined phase-2 iterations

_CACHE = {}


def _build_nc():
    import concourse.mybir as mybir
    import concourse.tile as tile
    from concourse import bacc

    f32 = mybir.dt.float32
    bf16 = mybir.dt.bfloat16

    nc = bacc.Bacc("TRN2", target_bir_lowering=False, debug=False,
                   num_devices=NCORES)

    hsT_d = nc.dram_tensor("hsT", [H, T], bf16, kind="ExternalInput")
    wq_d = nc.dram_tensor("wq", [128, KC, DC], bf16, kind="ExternalInput")
    wk_d = nc.dram_tensor("wk", [128, KC, DC], bf16, kind="ExternalInput")
    wv_d = nc.dram_tensor("wv", [128, KC, DC], bf16, kind="ExternalInput")
    wp_d = nc.dram_tensor("wp", [DC, H], bf16, kind="ExternalInput")
    bq_d = nc.dram_tensor("bq", [DC, 1], f32, kind="ExternalInput")
    bk_d = nc.dram_tensor("bk", [DC, 1], f32, kind="ExternalInput")
    mask_d = nc.dram_tensor("mask", [B, S], f32, kind="ExternalInput")
    out_d = nc.dram_tensor("out", [T, H], bf16, kind="ExternalOutput")

    i32 = mybir.dt.int32
    i16 = mybir.dt.int16
    EXP = mybir.ActivationFunctionType.Exp
    MULT = mybir.AluOpType.mult
    ADDOP = mybir.AluOpType.add
    # 1-term bf16 Schraudolph exp: pr_bf16 = bitcast_i16(round(s*A16 + B16)).
    # bf16 is the top 16 bits of f32, so the rounded i16 IS the bf16 prob —
    # ONE tensor_scalar per tile (the lone PSUM-evacuation op), no extra
    # engine work. Sawtooth rel err rms 1.8%, mean-free; on 3/16 key tiles
    # the ctx perturbation is ~0.9%. Offloads exp from ScalarE (the phase-2
    # pacer, saturated at ~17.2us/iter) onto DVE.
    SCH_A16 = 12102203.161561485 / 65536.0   # 2^7 * log2(e)
    SCH_B16 = 16248.576                      # 2^7 * (127 - 0.058)
    FAST_SET = (2, 7, 12)               # ILV tile indices on the fast path

    with tile.TileContext(nc) as tc:
        with (
            tc.tile_pool(name="const", bufs=1) as cpool,
            tc.tile_pool(name="qkv", bufs=1) as qpool,
            tc.tile_pool(name="probs", bufs=43) as ppool,
            tc.tile_pool(name="ctxn", bufs=2) as npool,
            tc.tile_pool(name="rcin", bufs=2) as ripool,
            tc.tile_pool(name="rec", bufs=2) as rpool,
            tc.tile_pool(name="ob", bufs=3) as opool,
            tc.tile_pool(name="ps_sc", bufs=2, space="PSUM") as pssc,
        ):
            wq_sb = cpool.tile([128, KC, DC], bf16)
            wk_sb = cpool.tile([128, KC, DC], bf16)
            wv_sb = cpool.tile([128, KC, DC], bf16)
            wp_sb = cpool.tile([DC, H], bf16)
            bq_sb = cpool.tile([DC, 1], f32)
            bk_sb = cpool.tile([DC, 1], f32)
            ones64 = cpool.tile([128, 64], bf16)
            mask_sb = cpool.tile([128, B, SKT], f32)
            em_sb = cpool.tile([128, B, SKT], f32)

            # ramp-critical DMAs only: wk+wq gate the first proj wave;
            # everything else is issued after the hs quarter-0 transfers.
            nc.sync.dma_start(wk_sb[:], wk_d.ap())
            nc.sync.dma_start(wq_sb[:], wq_d.ap())
            nc.vector.memset(ones64[:], 1.0)

            qT_sb = qpool.tile([128, T], bf16)   # rows 0:64 head0, 64:128 head1
            kT_sb = qpool.tile([128, T], bf16)
            # aug stationaries: per key tile g, [v'_h0 | em64 | v'_h1]
            aug_sb = qpool.tile([128, B * SKT, 192], bf16)

            # ---- phase-2 emission helpers ---------------------------------
            def emit_sc(i, skt, fast=False):
                b = i // SQB
                sq0 = b * S + (i % SQB) * 512
                sk = slice(b * S + skt * 128, b * S + skt * 128 + 128)
                sq = slice(sq0, sq0 + 512)
                sc_ps = pssc.tile([128, 1024], f32, tag="sc", name="sc_ps")
                nc.tensor.matmul(sc_ps[:, 0:512], lhsT=kT_sb[0:64, sk],
                                 rhs=qT_sb[0:64, sq], start=True, stop=True,
                                 skip_group_check=True)
                nc.tensor.matmul(sc_ps[:, 512:1024], lhsT=kT_sb[64:128, sk],
                                 rhs=qT_sb[64:128, sq], start=True, stop=True,
                                 skip_group_check=True)
                pr = ppool.tile([128, 1024], bf16, tag="pr", name="pr")
                if fast:
                    nc.vector.tensor_scalar(pr[:].bitcast(i16), sc_ps[:],
                                            SCH_A16, SCH_B16,
                                            op0=MULT, op1=ADDOP)
                else:
                    nc.scalar.activation(pr[:], sc_ps[:], EXP)
                return pr

            # ---- phase 1: QKV projection ----------------------------------
            with (
                tc.tile_pool(name="hs", bufs=1) as hpool,
                tc.tile_pool(name="ps1", bufs=1, space="PSUM") as ps1,
            ):
                hs_all = hpool.tile([128, KC, T], bf16)
                # quarter-chunk transfers, quarter-major: the first proj pass
                # (kT/qT blocks 0-1) only reads column-quarter 0 of every
                # chunk, so emitting all chunks' quarter-0 first lets scores
                # start after ~2 MB of input instead of all 8 MB.
                for q4 in range(4):
                    cs = slice(q4 * 1024, (q4 + 1) * 1024)
                    for k in range(KC):
                        nc.sync.dma_start(hs_all[:, k, cs],
                                          hsT_d.ap()[k * 128:(k + 1) * 128, cs])
                    if q4 == 0:
                        # deferred non-ramp-critical inputs, by first use:
                        # biases at the first proj writes, mask/em at the V
                        # phase, wp only at cproj
                        nc.sync.dma_start(bq_sb[:], bq_d.ap())
                        nc.sync.dma_start(bk_sb[:], bk_d.ap())
                        nc.sync.dma_start(
                            mask_sb[:],
                            mask_d.ap().rearrange("b (t p) -> p b t", p=128))
                        nc.scalar.activation(em_sb[:], mask_sb[:], EXP)
                    elif q4 == 1:
                        nc.sync.dma_start(wv_sb[:], wv_d.ap())
                    elif q4 == 2:
                        nc.sync.dma_start(wp_sb[:], wp_d.ap())

                # ---- globally smoothed score/exp emission -----------------
                # Every score window's matmul waits (sc-PSUM double buffer)
                # on exp(w-2), and a waiting matmul blocks the whole PE FIFO.
                # So score windows are metered into the PE stream at ~1 per
                # 1.1us of surrounding PE work (2-window batches: batch
                # member w+1 depends on exp(w-1) of the PREVIOUS batch, so
                # neither stalls), across phase 1 AND the ILVs. Phase 1's
                # PE-heavy stretch banks up to `CAP` windows of lookahead
                # which then absorbs the ILVs' structural deficit.
                CAP = 25                  # ppool(43) - 16 in-flight - margin
                probs_q = deque()
                cur = [0, 0]
                sched = {"credit": 2200.0, "allowed": 0}

                def _can_emit():
                    return (cur[0] < NIT and len(probs_q) < CAP
                            and cur[0] * SKT + cur[1] < sched["allowed"])

                def emit_one():
                    i, j = cur
                    fast = j in FAST_SET and i >= 1
                    probs_q.append(emit_sc(i, j, fast))
                    cur[:] = (i, j + 1) if j + 1 < SKT else (i + 1, 0)

                def tick(ns):
                    sched["credit"] += ns
                    while sched["credit"] >= 2200 and _can_emit():
                        emit_one()
                        if _can_emit():
                            emit_one()
                        sched["credit"] -= 2200
                    sched["credit"] = min(sched["credit"], 6600.0)

                def force_fill(n):
                    while len(probs_q) < n and cur[0] < NIT:
                        emit_one()

                def proj_joint(lanes):
                    """k-major accumulation over several (w, dst, bias, blk)
                    lanes at once — paces with arriving hs chunks."""
                    ps = [ps1.tile([128, 512], f32, tag=f"b{j}", name=f"ps_j{j}")
                          for j in range(len(lanes))]
                    for k in range(KC):
                        for j, (w_sb, _, _, blk) in enumerate(lanes):
                            cols = slice(blk * 512, (blk + 1) * 512)
                            nc.tensor.matmul(ps[j][:], lhsT=w_sb[:, k, :],
                                             rhs=hs_all[:, k, cols],
                                             start=(k == 0), stop=(k == KC - 1),
                                             skip_group_check=True)
                        tick(213 * len(lanes))
                    for j, (_, dst_sb, bias_sb, blk) in enumerate(lanes):
                        cols = slice(blk * 512, (blk + 1) * 512)
                        nc.vector.tensor_scalar_add(dst_sb[:, cols], ps[j][:],
                                                    bias_sb[:, 0:1])

                KL = (wk_sb, kT_sb, bk_sb)
                QL = (wq_sb, qT_sb, bq_sb)
                # batch-0 K and Q first; score emission gates open per wave as
                # the needed kT/qT blocks land.
                proj_joint([KL + (0,), KL + (1,), QL + (0,)])
                sched["allowed"] = 8          # iter 0, keys 0:1024
                proj_joint([KL + (2,), KL + (3,), QL + (1,)])
                sched["allowed"] = 32         # iters 0-1 (batch-0 kT done)
                proj_joint([QL + (2,), QL + (3,)])
                sched["allowed"] = 64         # iters 2-3
                proj_joint([KL + (4,), KL + (5,), QL + (4,), QL + (5,)])
                proj_joint([KL + (6,), KL + (7,), QL + (6,), QL + (7,)])
                sched["allowed"] = NIT * SKT  # everything

                # V natural per key tile g; PSUM->aug copy fuses em scaling.
                # em columns for all 32 tiles in one broadcast op (em is
                # per-(partition, tile); replicate across the 64 cols)
                em_bc = em_sb[:].rearrange("p b t -> p (b t)").unsqueeze(2)
                nc.vector.tensor_scalar_mul(
                    aug_sb[:, :, 64:128], em_bc.broadcast_to([128, B * SKT, 64]),
                    1.0)
                for g in range(B * SKT):
                    b, skt = g // SKT, g % SKT
                    em = em_sb[:, b, skt:skt + 1]
                    gc = slice(g * 128, (g + 1) * 128)
                    v_ps = ps1.tile([128, DC], f32, tag=f"b{g % 4}", name="v_ps")
                    for k in range(KC):
                        nc.tensor.matmul(v_ps[:], lhsT=hs_all[:, k, gc],
                                         rhs=wv_sb[:, k, :],
                                         start=(k == 0), stop=(k == KC - 1),
                                         skip_group_check=True)
                    # both 64-wide v halves in one op: out cols {0:64,128:192}
                    aug_v = aug_sb[:, g, :].rearrange("p (a b) -> p a b", a=3)[:, 0:3:2, :]
                    src_v = v_ps[:].rearrange("p (two c) -> p two c", two=2)
                    nc.vector.tensor_scalar_mul(aug_v, src_v, em)
                    tick(650)

            # ---- phase 2: attention + c_proj ------------------------------
            with tc.tile_pool(name="ps_ab", bufs=2, space="PSUM") as psab:
                prev = None  # (ctxn tile, sq0) pending c_proj

                def emit_cproj_chunks(ctxn, sq0, t4s, final=False):
                    # output DMA rides the idle GpSimd queue so it cannot
                    # delay the sync-queue rec_in realign DMAs (normalize
                    # critical path)
                    for t4 in t4s:
                        tok = slice(t4 * 128, (t4 + 1) * 128)
                        rows = slice(sq0 + t4 * 128, sq0 + (t4 + 1) * 128)
                        op_a = psab.tile([128, 512], f32, tag="pa", name="op_a")
                        op_b = psab.tile([128, 512], f32, tag="pb", name="op_b")
                        nc.tensor.matmul(op_a[:], lhsT=ctxn[:, tok],
                                         rhs=wp_sb[:, 0:512], start=True, stop=True,
                                         skip_group_check=True)
                        nc.tensor.matmul(op_b[:], lhsT=ctxn[:, tok],
                                         rhs=wp_sb[:, 512:1024], start=True, stop=True,
                                         skip_group_check=True)
                        ob = opool.tile([128, 1024], bf16, tag="ob", name="ob")
                        nc.vector.tensor_copy(ob[:, 0:512], op_a[:])
                        if final:
                            # exp stream is over: ScalarE is idle, split the
                            # PSUM evacuation so the tail isn't DVE-serial
                            nc.scalar.copy(ob[:, 512:1024], op_b[:])
                        else:
                            nc.vector.tensor_copy(ob[:, 512:1024], op_b[:])
                        nc.gpsimd.dma_start(out_d.ap()[rows, :], ob[:])
                        tick(440)

                force_fill(SKT)
                probs = [probs_q.popleft() for _ in range(SKT)]
                for i in range(NIT):
                    b = i // SQB
                    sq0 = b * S + (i % SQB) * 512
                    # --- ILV: aug matmuls of i, with metered lookahead
                    # score/exp emission and the previous iteration's c_proj
                    # spread through (2 chunks at skt 4, 2 at skt 10) -----
                    psA = psab.tile([128, 512], f32, tag="pa", name="psA")
                    psB = psab.tile([128, 512], f32, tag="pb", name="psB")
                    for skt in range(SKT):
                        g = b * SKT + skt
                        st, sp = (skt == 0), (skt == SKT - 1)
                        nc.tensor.matmul(psA[:], lhsT=aug_sb[:, g, 0:128],
                                         rhs=probs[skt][:, 0:512], start=st, stop=sp,
                                         skip_group_check=True)
                        nc.tensor.matmul(psB[:], lhsT=aug_sb[:, g, 64:192],
                                         rhs=probs[skt][:, 512:1024], start=st, stop=sp,
                                         skip_group_check=True)
                        tick(440)
                        # single c_proj chunks spread through the ILV: chunk
                        # t+1's matmul reuses chunk t's PSUM bank and so waits
                        # on its CAST — spacing them 3 skt apart hides that
                        # behind aug work instead of stalling the PE FIFO
                        if prev is not None and skt in (4, 7, 10, 13):
                            emit_cproj_chunks(prev[0], prev[1],
                                              ((skt - 4) // 3,))
                            if skt == 13:
                                prev = None
                    # --- N: normalize --------------------------------------
                    se_st = ripool.tile([128, 512], f32, tag="st", name="se_st")
                    nc.vector.tensor_copy(se_st[64:128, :], psA[64:128, :])
                    nc.vector.tensor_copy(se_st[0:64, :], psB[0:64, :])
                    rec_in = ripool.tile([128, 512], f32, tag="ri", name="rec_in")
                    nc.sync.dma_start(rec_in[0:64, :], se_st[64:128, :])
                    nc.sync.dma_start(rec_in[64:128, :], se_st[0:64, :])
                    rec = rpool.tile([128, 512], f32, tag="rc", name="rec")
                    nc.vector.reciprocal_approx_fast(rec[:], rec_in[:])
                    ctxn = npool.tile([128, 512], bf16, tag="cn", name="ctxn")
                    nc.vector.tensor_tensor(ctxn[0:64, :], psA[0:64, :],
                                            rec[0:64, :], op=MULT)
                    nc.vector.tensor_tensor(ctxn[64:128, :], psB[64:128, :],
                                            rec[64:128, :], op=MULT)
                    # --- C(i-1), if not already emitted mid-ILV ------------
                    if prev is not None:
                        emit_cproj_chunks(prev[0], prev[1], (0, 1, 2, 3))
                    prev = (ctxn, sq0)
                    if i + 1 < NIT:
                        force_fill(SKT)  # top up if the metering ran dry
                        probs = [probs_q.popleft() for _ in range(SKT)]

                emit_cproj_chunks(prev[0], prev[1], (0, 1, 2, 3), final=True)

    nc.compile()
    return nc


def _enable_ldw_opt():
    """No-op: --enable-ldw-opt=true crashes walrus codegen
    (CoreV3GenImpl visitInstLdweights) on this kernel's LDWEIGHTS mix."""
    return


def _get_nc():
    if "nc" not in _CACHE:
        _enable_ldw_opt()
        _CACHE["nc"] = _build_nc()
    return _CACHE["nc"]


def kernel(hidden_states, attention_mask, c_attn_w, c_attn_b, c_proj_w, c_proj_b):
    from concourse.bass_utils import run_bass_kernel_spmd

    bf16 = ml_dtypes.bfloat16
    hs = np.asarray(hidden_states, dtype=np.float32).reshape(T, H)
    hsT = np.ascontiguousarray(hs.T).astype(bf16)
    mask = np.ascontiguousarray(
        np.broadcast_to(
            np.asarray(attention_mask, dtype=np.float32).reshape(B, 1, 1, S)[:, 0, 0, :],
            (B, S),
        )
    )
    w = np.asarray(c_attn_w, dtype=np.float32)
    bqkv = np.asarray(c_attn_b, dtype=np.float32)
    wp_full = np.asarray(c_proj_w, dtype=np.float32)
    scale = 1.0 / np.sqrt(HD)

    def pack(a):  # [H, DC] -> [128, KC, DC], contiguous per-partition lines
        return np.ascontiguousarray(
            a.reshape(KC, 128, DC).transpose(1, 0, 2)).astype(bf16)

    in_maps = []
    for c in range(NCORES):
        lo, hi = c * DC, (c + 1) * DC
        in_maps.append({
            "hsT": hsT,
            "wq": pack(w[:, lo:hi] * scale),
            "wk": pack(w[:, H + lo:H + hi]),
            "wv": pack(w[:, 2 * H + lo:2 * H + hi]),
            "wp": np.ascontiguousarray(wp_full[lo:hi, :]).astype(bf16),
            "bq": np.ascontiguousarray((bqkv[lo:hi] * scale).reshape(DC, 1)),
            "bk": np.ascontiguousarray(bqkv[H + lo:H + hi].reshape(DC, 1)),
            "mask": mask,
        })

    res = run_bass_kernel_spmd(_get_nc(), in_maps, core_ids=list(range(NCORES)))
    _CACHE["last_result"] = res
    acc = np.zeros((T, H), dtype=np.float32)
    for c in range(NCORES):
        acc += np.asarray(res.results[c]["out"], dtype=np.float32)
    # v-bias contributes the constant row bv @ c_proj_w (exact, host-side)
    bv_full = bqkv[2 * H:3 * H]
    acc += (bv_full @ wp_full + np.asarray(c_proj_b, dtype=np.float32))[None, :]
    return acc.reshape(B, S, H)

